# revision 4
# baseline (speedup 1.0000x reference)
"""Multi-class DICE loss on 8 Trainium2 NeuronCores.

Reference computation (B=16, C=8, H=W=512):
    onehot = (mask[:,None] == arange(C))        # [B,C,H,W]
    num  = sum(output * onehot, axis=(2,3))     # [B,C]
    den1 = sum(output * output, axis=(2,3))     # [B,C]
    den2 = sum(onehot, axis=(2,3))              # [B,C]
    dice = 2 * (num + eps) / (den1 + den2 + eps)
    loss = 1 - sum(dice) / (B*B)

Sharding: pure data parallel over batch; each of 8 cores takes 2
samples (16 (b,c) class-tiles of [128, 2048]).

v2 layout (DMA-bound target ~53us/core at 358 GB/s for 18 MiB):
  x arrives via SWDGE (gpsimd) cast-DMA f32->bf16, so every per-class
  elementwise op runs in a 16-bit DVE perf mode:
    DVE  scalar_tensor_tensor (mask==c)*x bf16 2x mode, accum -> p_num
    DVE  tensor_scalar (mask==c) bf16 4x mode, accum       -> p_den2
    ACT  activation Square (bf16 in), accum                -> p_den1
  No PE mini-reduces per class (the baseline's 64 matmuls + 32 ACT
  mini-reduce ops made ACT/PE the bottleneck at ~65us busy).
  The int32 mask converts to bf16 labels once per sample on DVE.
  Tail: three ones-matmul folds [128,16]->[1,16] into one PSUM bank,
  dice evaluated on partition 0, per-core dice-sum written out. The 8
  per-core partials are summed on the host (the unshard step).
"""

import os
from contextlib import ExitStack

import numpy as np

import concourse.bacc as bacc
import concourse.bass as bass
import concourse.tile as tile
from concourse import mybir
from concourse.bass_utils import run_bass_kernel_spmd

N_CORES = 8
B, C, H, W = 16, 8, 512, 512
B_LOC = B // N_CORES          # samples per core
HWPIX = H * W                 # 262144 pixels per (b, c)
P = 128                       # SBUF partitions
NCOL = HWPIX // P             # 2048 free-dim columns per class-tile
ROWS = B_LOC * C              # 16 (b, c) pairs per core
G = 4                         # max classes per x DMA group
EPS = 1e-7


_cache: dict = {}
last_results = None           # BassKernelResults of the most recent run


def _build(mask64: bool) -> bass.Bass:
    nc = bacc.Bacc(
        "TRN2",
        target_bir_lowering=False,
        debug=False,
        num_devices=1,
    )
    f32 = mybir.dt.float32
    bf16 = mybir.dt.bfloat16
    i32 = mybir.dt.int32

    x = nc.dram_tensor("x", [ROWS, P, NCOL], f32, kind="ExternalInput")
    # int64 masks arrive as little-endian int32 pairs; low word holds the
    # label, extracted with a stride-2 access pattern on chip.
    m_cols = NCOL * 2 if mask64 else NCOL
    m = nc.dram_tensor("m", [B_LOC, P, m_cols], i32, kind="ExternalInput")
    part = nc.dram_tensor("part", [1], f32, kind="ExternalOutput")
    # Per-(b,c) partials for debugging: [num(16) | den1(16) | den2(16)].
    dbg = nc.dram_tensor("dbg", [48], f32, kind="ExternalOutput")

    with tile.TileContext(nc) as tc, ExitStack() as ctx:
        xpool = ctx.enter_context(tc.tile_pool(name="xp", bufs=6))
        mpool = ctx.enter_context(tc.tile_pool(name="mp", bufs=2))
        mfpool = ctx.enter_context(tc.tile_pool(name="mfp", bufs=2))
        jpool = ctx.enter_context(tc.tile_pool(name="jp", bufs=2))
        spool = ctx.enter_context(tc.tile_pool(name="sp", bufs=2))
        acc = ctx.enter_context(tc.tile_pool(name="acc", bufs=1))
        pspool = ctx.enter_context(tc.tile_pool(name="ps", bufs=1, space="PSUM"))

        ones32 = acc.tile([P, 1], f32, tag="ones32")
        nc.vector.memset(ones32, 1.0)

        # Per-partition partial sums, one column per (b, c) pair.
        # Separate tiles per writing engine so DVE and ACT accumulator
        # writes never cross-serialize.
        p_num = acc.tile([P, ROWS], f32, tag="p_num")
        p_den1 = acc.tile([P, ROWS], f32, tag="p_den1")
        p_den2 = acc.tile([P, ROWS], f32, tag="p_den2")

        for b in range(B_LOC):
            mraw = mpool.tile([P, m_cols], i32, tag="mraw")
            nc.sync.dma_start(out=mraw, in_=m[b])
            if mask64:
                msrc = mraw.rearrange("p (n two) -> p n two", two=2)[:, :, 0]
            else:
                msrc = mraw[:]
            # int32 -> bf16 label copy on DVE; labels 0..7 are exact.
            mf = mfpool.tile([P, NCOL], bf16, tag="mf")
            nc.vector.tensor_copy(out=mf, in_=msrc)

            # The first sample's x tiles arrive staircased (1, 3, 4
            # classes) so compute starts as soon as 1 MB has been read;
            # the last group is 1 class so the post-DMA compute tail is
            # a single class-tile.
            groups = [1, 3, G] if b == 0 else [G, 3, 1]
            c0 = 0
            for gsz in groups:
                xt = xpool.tile([P, G, NCOL], bf16, tag="xt")
                # SWDGE cast-DMA: reads f32 from HBM, writes bf16 to SBUF.
                nc.gpsimd.dma_start(
                    out=xt[:, 0:gsz, :],
                    in_=x[b * C + c0 : b * C + c0 + gsz].transpose([1, 0, 2]),
                )
                for i in range(gsz):
                    c = c0 + i
                    col = b * C + c
                    # num partial: (mask == c) * x, accumulated per partition
                    junk = jpool.tile([P, NCOL], bf16, tag="jd")
                    nc.vector.scalar_tensor_tensor(
                        out=junk,
                        in0=mf,
                        scalar=float(c),
                        in1=xt[:, i, :],
                        op0=mybir.AluOpType.is_equal,
                        op1=mybir.AluOpType.mult,
                        accum_out=p_num[:, col : col + 1],
                    )
                    # den2 partial: (mask == c), accumulated per partition.
                    # With accum_out this lowers to TensorScalarPtrReduce,
                    # which requires op1 (the add-fold).
                    ejunk = jpool.tile([P, NCOL], bf16, tag="je")
                    nc.vector.tensor_scalar(
                        out=ejunk,
                        in0=mf,
                        scalar1=float(c),
                        scalar2=0.0,
                        op0=mybir.AluOpType.is_equal,
                        op1=mybir.AluOpType.add,
                        accum_out=p_den2[:, col : col + 1],
                    )
                    # den1 partial: x^2, accumulated per partition
                    sjunk = spool.tile([P, NCOL], mybir.dt.float8e4, tag="ja")
                    nc.scalar.activation(
                        out=sjunk,
                        in_=xt[:, i, :],
                        func=mybir.ActivationFunctionType.Square,
                        accum_out=p_den1[:, col : col + 1],
                    )
                c0 += gsz

        # Fold partition dim: [128, 16] -> one PSUM bank [1, 48].
        ps = pspool.tile([1, 48], f32, tag="ps")
        nc.tensor.matmul(out=ps[:, 0:16], lhsT=ones32, rhs=p_num[:], start=True, stop=True)
        nc.tensor.matmul(out=ps[:, 16:32], lhsT=ones32, rhs=p_den1[:], start=True, stop=True)
        nc.tensor.matmul(out=ps[:, 32:48], lhsT=ones32, rhs=p_den2[:], start=True, stop=True)

        # PSUM -> SBUF once (an op may read at most one PSUM input);
        # also serves as the debug-partials DMA source.
        dbgs = acc.tile([1, 48], f32, tag="dbgs")
        nc.vector.tensor_copy(out=dbgs, in_=ps[:])
        nc.sync.dma_start(out=dbg[:], in_=dbgs)

        # dice = (num + eps) / (den1 + den2 + eps) on partition 0;
        # the factor 2 and the 1 - .../B^2 affine are applied on host.
        dene = acc.tile([1, ROWS], f32, tag="dene")
        nc.vector.scalar_tensor_tensor(
            out=dene,
            in0=dbgs[:, 16:32],
            scalar=EPS,
            in1=dbgs[:, 32:48],
            op0=mybir.AluOpType.add,
            op1=mybir.AluOpType.add,
        )
        rec = acc.tile([1, ROWS], f32, tag="rec")
        nc.vector.reciprocal(out=rec, in_=dene)
        nume = acc.tile([1, ROWS], f32, tag="nume")
        nc.vector.tensor_scalar_add(out=nume, in0=dbgs[:, 0:16], scalar1=EPS)
        dice = acc.tile([1, ROWS], f32, tag="dice")
        nc.vector.tensor_tensor(
            out=dice, in0=nume, in1=rec, op=mybir.AluOpType.mult
        )
        lsum = acc.tile([1, 1], f32, tag="lsum")
        nc.vector.tensor_reduce(
            out=lsum, in_=dice, axis=mybir.AxisListType.X, op=mybir.AluOpType.add
        )
        nc.sync.dma_start(out=part[:], in_=lsum)

    nc.compile()
    return nc


def _get(mask64: bool) -> bass.Bass:
    if mask64 not in _cache:
        _cache[mask64] = _build(mask64)
    return _cache[mask64]


def make_in_maps(output: np.ndarray, mask: np.ndarray, mask64: bool):
    in_maps = []
    for i in range(N_CORES):
        xs = output[i * B_LOC : (i + 1) * B_LOC].reshape(ROWS, P, NCOL)
        ms = np.ascontiguousarray(mask[i * B_LOC : (i + 1) * B_LOC])
        if mask64:
            ms = ms.view(np.int32).reshape(B_LOC, P, NCOL * 2)
        else:
            ms = ms.reshape(B_LOC, P, NCOL)
        in_maps.append({"x": np.ascontiguousarray(xs), "m": ms})
    return in_maps


def kernel(output: np.ndarray, mask: np.ndarray) -> np.ndarray:
    global last_results
    output = np.ascontiguousarray(np.asarray(output, dtype=np.float32))
    mask = np.asarray(mask)
    assert output.shape == (B, C, H, W), output.shape
    assert mask.shape == (B, H, W), mask.shape
    mask64 = mask.dtype.itemsize == 8
    if not mask64 and mask.dtype != np.int32:
        mask = mask.astype(np.int32)

    nc = _get(mask64)
    in_maps = make_in_maps(output, mask, mask64)
    last_results = run_bass_kernel_spmd(
        nc,
        in_maps,
        list(range(N_CORES)),
        trace=bool(os.environ.get("DICE_TRACE")),
    )
    total = 0.0
    for r in last_results.results:
        total += float(np.asarray(r["part"]).reshape(()))
    loss = 1.0 - 2.0 * total / (B * B)
    return np.float32(loss).reshape(())


# revision 7
# speedup vs baseline: 1.2153x; 1.2153x over previous
"""Multi-class DICE loss on 8 Trainium2 NeuronCores.

Reference computation (B=16, C=8, H=W=512):
    onehot = (mask[:,None] == arange(C))        # [B,C,H,W]
    num  = sum(output * onehot, axis=(2,3))     # [B,C]
    den1 = sum(output * output, axis=(2,3))     # [B,C]
    den2 = sum(onehot, axis=(2,3))              # [B,C]
    dice = 2 * (num + eps) / (den1 + den2 + eps)
    loss = 1 - sum(dice) / (B*B)

Sharding: pure data parallel over batch; each of 8 cores takes 2
samples (16 (b,c) class-tiles of [128, 2048]).

v3 layout. DMA floor is ~53us/core (18.9 MB at 358 GB/s); the goal is
to keep every engine below that. DVE accumulate ops always run in 1x
mode (~2.16us per [128,2048] tile regardless of dtype), so only num
uses one; den2 avoids a second 1x pass:
  DVE  scalar_tensor_tensor (mask==c)*x f32, accum     -> p_num col
  ACT  activation Square f32, accum                    -> p_den1 col
  DVE  tensor_scalar (mask==c) bf16 4x mode, NO accum  -> eq tile
  PE   onesT @ eq chunk-matmuls accumulate             -> ps2[col, 0:512]
den2's free-dim fold happens ONCE at the end: tensor_reduce over the
[16, 512] PSUM tile (all 16 (b,c) rows reduce in parallel).
The kernel outputs raw partials (num|den1 [32], den2 [16]); the dice
ratio + the 1 - 2*sum/B^2 affine run on host during the unshard step
(the sharding_hint's all-reduce of per-sample per-class partials).
"""

import os
from contextlib import ExitStack

import numpy as np

import concourse.bacc as bacc
import concourse.bass as bass
import concourse.tile as tile
from concourse import mybir
from concourse.bass_utils import run_bass_kernel_spmd

N_CORES = 8
B, C, H, W = 16, 8, 512, 512
B_LOC = B // N_CORES          # samples per core
HWPIX = H * W                 # 262144 pixels per (b, c)
P = 128                       # SBUF partitions
NCOL = HWPIX // P             # 2048 free-dim columns per class-tile
ROWS = B_LOC * C              # 16 (b, c) pairs per core
G = 4                         # max classes per x DMA group
NCHUNK = 4                    # eq matmul chunks per class-tile
CHUNK = NCOL // NCHUNK        # 512
EPS = 1e-7


_cache: dict = {}
last_results = None           # BassKernelResults of the most recent run


def _build(mask64: bool) -> bass.Bass:
    nc = bacc.Bacc(
        "TRN2",
        target_bir_lowering=False,
        debug=False,
        num_devices=1,
    )
    f32 = mybir.dt.float32
    bf16 = mybir.dt.bfloat16
    i32 = mybir.dt.int32

    x = nc.dram_tensor("x", [ROWS, P, NCOL], f32, kind="ExternalInput")
    # int64 masks arrive as little-endian int32 pairs; low word holds the
    # label, extracted with a stride-2 access pattern on chip.
    m_cols = NCOL * 2 if mask64 else NCOL
    m = nc.dram_tensor("m", [B_LOC, P, m_cols], i32, kind="ExternalInput")
    nd = nc.dram_tensor("nd", [2 * ROWS], f32, kind="ExternalOutput")
    d2 = nc.dram_tensor("d2", [ROWS], f32, kind="ExternalOutput")

    with tile.TileContext(nc) as tc, ExitStack() as ctx:
        xpool = ctx.enter_context(tc.tile_pool(name="xp", bufs=4))
        mpool = ctx.enter_context(tc.tile_pool(name="mp", bufs=2))
        mfpool = ctx.enter_context(tc.tile_pool(name="mfp", bufs=2))
        epool = ctx.enter_context(tc.tile_pool(name="ep", bufs=3))
        jpool = ctx.enter_context(tc.tile_pool(name="jp", bufs=2))
        spool = ctx.enter_context(tc.tile_pool(name="sp", bufs=2))
        acc = ctx.enter_context(tc.tile_pool(name="acc", bufs=1))
        pspool = ctx.enter_context(tc.tile_pool(name="ps", bufs=1, space="PSUM"))

        ones32 = acc.tile([P, 1], f32, tag="ones32")
        nc.vector.memset(ones32, 1.0)
        # One-hot lhsT matrices: etab[col][:, m] = 1 iff m == col. A
        # matmul with lhsT=etab[col] deposits the partition-fold of its
        # rhs into PSUM row `col` and exact zeros elsewhere (PE output
        # base-partition must be 0/32/64, so rows can't be addressed via
        # the out AP). Built on GpSimd, which is otherwise idle.
        etab = []
        for col in range(ROWS):
            e = acc.tile([P, ROWS], bf16, tag=f"e{col}")
            nc.gpsimd.memset(e, 0.0)
            nc.gpsimd.memset(e[:, col : col + 1], 1.0)
            etab.append(e)

        # Per-partition partial sums, one column per (b, c) pair.
        # Separate tiles per writing engine so DVE and ACT accumulator
        # writes never cross-serialize.
        p_num = acc.tile([P, ROWS], f32, tag="p_num")
        p_den1 = acc.tile([P, ROWS], f32, tag="p_den1")
        # den2: one PSUM row per (b, c), chunk-accumulated by PE.
        ps2 = pspool.tile([ROWS, CHUNK], f32, tag="ps2")

        for b in range(B_LOC):
            mraw = mpool.tile([P, m_cols], i32, tag="mraw")
            nc.sync.dma_start(out=mraw, in_=m[b])
            if mask64:
                msrc = mraw.rearrange("p (n two) -> p n two", two=2)[:, :, 0]
            else:
                msrc = mraw[:]
            # int32 -> bf16 label copy on DVE; labels 0..7 are exact.
            mf = mfpool.tile([P, NCOL], bf16, tag="mf")
            nc.vector.tensor_copy(out=mf, in_=msrc)

            # The first sample's x tiles arrive staircased (1, 3, 4
            # classes) so compute starts as soon as 1 MB has landed; the
            # last group is 1 class so the post-DMA compute tail is short.
            groups = [1, 3, G] if b == 0 else [G, 3, 1]
            c0 = 0
            for gsz in groups:
                xt = xpool.tile([P, G, NCOL], f32, tag="xt")
                nc.sync.dma_start(
                    out=xt[:, 0:gsz, :],
                    in_=x[b * C + c0 : b * C + c0 + gsz].transpose([1, 0, 2]),
                )
                for i in range(gsz):
                    c = c0 + i
                    col = b * C + c
                    # num partial: (mask == c) * x, accumulated per partition
                    junk = jpool.tile([P, NCOL], mybir.dt.float8e4, tag="jd")
                    nc.vector.scalar_tensor_tensor(
                        out=junk,
                        in0=mf,
                        scalar=float(c),
                        in1=xt[:, i, :],
                        op0=mybir.AluOpType.is_equal,
                        op1=mybir.AluOpType.mult,
                        accum_out=p_num[:, col : col + 1],
                    )
                    # den1 partial: x^2, accumulated per partition
                    sjunk = spool.tile([P, NCOL], mybir.dt.float8e4, tag="ja")
                    nc.scalar.activation(
                        out=sjunk,
                        in_=xt[:, i, :],
                        func=mybir.ActivationFunctionType.Square,
                        accum_out=p_den1[:, col : col + 1],
                    )
                    # den2: eq = (mask == c) in bf16 (DVE 4x mode), then
                    # PE folds partitions into ps2 row `col` via the
                    # one-hot lhsT, one accumulation chain over all 64
                    # matmuls (other rows receive exact zeros).
                    eq = epool.tile([P, NCOL], bf16, tag="eq")
                    nc.vector.tensor_scalar(
                        out=eq,
                        in0=mf,
                        scalar1=float(c),
                        scalar2=None,
                        op0=mybir.AluOpType.is_equal,
                    )
                    for j in range(NCHUNK):
                        nc.tensor.matmul(
                            out=ps2[:, :],
                            lhsT=etab[col],
                            rhs=eq[:, j * CHUNK : (j + 1) * CHUNK],
                            start=(col == 0 and j == 0),
                            stop=(col == ROWS - 1 and j == NCHUNK - 1),
                        )
                c0 += gsz

        # Fold partition dim of num/den1: [128, 16] -> PSUM [1, 32].
        ps = pspool.tile([1, 2 * ROWS], f32, tag="ps")
        nc.tensor.matmul(out=ps[:, 0:ROWS], lhsT=ones32, rhs=p_num[:], start=True, stop=True)
        nc.tensor.matmul(out=ps[:, ROWS:], lhsT=ones32, rhs=p_den1[:], start=True, stop=True)
        nds = acc.tile([1, 2 * ROWS], f32, tag="nds")
        nc.vector.tensor_copy(out=nds, in_=ps[:])
        nc.sync.dma_start(out=nd[:], in_=nds)

        # den2: fold the chunk dim of all 16 rows at once.
        d2col = acc.tile([ROWS, 1], f32, tag="d2col")
        nc.vector.tensor_reduce(
            out=d2col, in_=ps2[:], axis=mybir.AxisListType.X, op=mybir.AluOpType.add
        )
        nc.sync.dma_start(out=d2[:], in_=d2col)

    nc.compile()
    return nc


def _get(mask64: bool) -> bass.Bass:
    if mask64 not in _cache:
        _cache[mask64] = _build(mask64)
    return _cache[mask64]


def make_in_maps(output: np.ndarray, mask: np.ndarray, mask64: bool):
    in_maps = []
    for i in range(N_CORES):
        xs = output[i * B_LOC : (i + 1) * B_LOC].reshape(ROWS, P, NCOL)
        ms = np.ascontiguousarray(mask[i * B_LOC : (i + 1) * B_LOC])
        if mask64:
            ms = ms.view(np.int32).reshape(B_LOC, P, NCOL * 2)
        else:
            ms = ms.reshape(B_LOC, P, NCOL)
        in_maps.append({"x": np.ascontiguousarray(xs), "m": ms})
    return in_maps


def kernel(output: np.ndarray, mask: np.ndarray) -> np.ndarray:
    global last_results
    output = np.ascontiguousarray(np.asarray(output, dtype=np.float32))
    mask = np.asarray(mask)
    assert output.shape == (B, C, H, W), output.shape
    assert mask.shape == (B, H, W), mask.shape
    mask64 = mask.dtype.itemsize == 8
    if not mask64 and mask.dtype != np.int32:
        mask = mask.astype(np.int32)

    nc = _get(mask64)
    in_maps = make_in_maps(output, mask, mask64)
    last_results = run_bass_kernel_spmd(
        nc,
        in_maps,
        list(range(N_CORES)),
        trace=bool(os.environ.get("DICE_TRACE")),
    )
    # Unshard: dice over the gathered per-(b,c) partials, then the
    # 1 - 2*sum/B^2 affine.
    total = 0.0
    for r in last_results.results:
        nd_ = np.asarray(r["nd"], dtype=np.float64).reshape(2, ROWS)
        d2_ = np.asarray(r["d2"], dtype=np.float64).reshape(ROWS)
        num, den1 = nd_[0], nd_[1]
        total += float(np.sum((num + EPS) / (den1 + d2_ + EPS)))
    loss = 1.0 - 2.0 * total / (B * B)
    return np.float32(loss).reshape(())


# revision 8
# speedup vs baseline: 1.3202x; 1.0863x over previous
"""Multi-class DICE loss on 8 Trainium2 NeuronCores.

Reference computation (B=16, C=8, H=W=512):
    onehot = (mask[:,None] == arange(C))        # [B,C,H,W]
    num  = sum(output * onehot, axis=(2,3))     # [B,C]
    den1 = sum(output * output, axis=(2,3))     # [B,C]
    den2 = sum(onehot, axis=(2,3))              # [B,C]
    dice = 2 * (num + eps) / (den1 + den2 + eps)
    loss = 1 - sum(dice) / (B*B)

Sharding: pure data parallel over batch; each of 8 cores takes 2
samples (16 (b,c) class-tiles of [128, 2048]).

v4 layout. The DMA stream (16 MiB x f32 + 0.5 MiB u8 masks at ~330+
GB/s) is the target critical path; every engine stays under it:
  DVE  scalar_tensor_tensor (mask==c)*x f32, accum -> p_num col
       (2.28us/class; DVE accumulate ops always run 1x)
  DVE  tensor_scalar eq=(mask==c) bf16 4x, NO accum (0.69us/class),
       emitted per-sample BEFORE the x loop: eq depends only on the
       mask, so these fill DVE's head gap while x DMAs stream, and
       mid-stream DVE carries only the stt (2.36 < 2.93us DMA/class).
  ACT  Square x f32, accum -> p_den1 col; also the u8->bf16 mask cast
  PE   one-hot-lhsT matmuls fold eq partitions -> ps2[col, 0:512],
       one 64-matmul PSUM accumulation chain (off-row adds are 0)
The mask ships as uint8 (values 0..7, lossless) to cut its DMA 4x.
den2's free-dim fold is a single tensor_reduce of ps2 [16,512] at the
end. The kernel outputs raw partials (num|den1 [32], den2 [16]); the
dice ratio and the 1 - 2*sum/B^2 affine run on host in the unshard
step (the sharding_hint's all-reduce of per-class partials).
"""

import os
from contextlib import ExitStack

import numpy as np

import concourse.bacc as bacc
import concourse.bass as bass
import concourse.tile as tile
from concourse import mybir
from concourse.bass_utils import run_bass_kernel_spmd

N_CORES = 8
B, C, H, W = 16, 8, 512, 512
B_LOC = B // N_CORES          # samples per core
HWPIX = H * W                 # 262144 pixels per (b, c)
P = 128                       # SBUF partitions
NCOL = HWPIX // P             # 2048 free-dim columns per class-tile
ROWS = B_LOC * C              # 16 (b, c) pairs per core
G = 4                         # max classes per x DMA group
NCHUNK = 4                    # eq matmul chunks per class-tile
CHUNK = NCOL // NCHUNK        # 512
EPS = 1e-7


_cache: dict = {}
last_results = None           # BassKernelResults of the most recent run


def _build() -> bass.Bass:
    nc = bacc.Bacc(
        "TRN2",
        target_bir_lowering=False,
        debug=False,
        num_devices=1,
    )
    f32 = mybir.dt.float32
    bf16 = mybir.dt.bfloat16
    u8 = mybir.dt.uint8

    x = nc.dram_tensor("x", [ROWS, P, NCOL], f32, kind="ExternalInput")
    m = nc.dram_tensor("m", [B_LOC, P, NCOL], u8, kind="ExternalInput")
    nd = nc.dram_tensor("nd", [2 * ROWS], f32, kind="ExternalOutput")
    d2 = nc.dram_tensor("d2", [ROWS], f32, kind="ExternalOutput")

    with tile.TileContext(nc) as tc, ExitStack() as ctx:
        xpool = ctx.enter_context(tc.tile_pool(name="xp", bufs=5))
        mpool = ctx.enter_context(tc.tile_pool(name="mp", bufs=2))
        mfpool = ctx.enter_context(tc.tile_pool(name="mfp", bufs=2))
        epool = ctx.enter_context(tc.tile_pool(name="ep", bufs=3))
        jpool = ctx.enter_context(tc.tile_pool(name="jp", bufs=2))
        spool = ctx.enter_context(tc.tile_pool(name="sp", bufs=2))
        acc = ctx.enter_context(tc.tile_pool(name="acc", bufs=1))
        pspool = ctx.enter_context(tc.tile_pool(name="ps", bufs=1, space="PSUM"))

        ones32 = acc.tile([P, 1], f32, tag="ones32")
        nc.vector.memset(ones32, 1.0)
        # One-hot lhsT matrices: etab[col][:, m] = 1 iff m == col. A
        # matmul with lhsT=etab[col] deposits the partition-fold of its
        # rhs into PSUM row `col` and exact zeros elsewhere (PE output
        # base-partition must be 0/32/64, so rows can't be addressed via
        # the out AP). Built on GpSimd, which is otherwise idle.
        etab = []
        for col in range(ROWS):
            e = acc.tile([P, ROWS], bf16, tag=f"e{col}")
            nc.gpsimd.memset(e, 0.0)
            nc.gpsimd.memset(e[:, col : col + 1], 1.0)
            etab.append(e)

        # Per-partition partial sums, one column per (b, c) pair.
        # Separate tiles per writing engine so DVE and ACT accumulator
        # writes never cross-serialize.
        p_num = acc.tile([P, ROWS], f32, tag="p_num")
        p_den1 = acc.tile([P, ROWS], f32, tag="p_den1")
        # den2: one PSUM row per (b, c), chunk-accumulated by PE.
        ps2 = pspool.tile([ROWS, CHUNK], f32, tag="ps2")

        for b in range(B_LOC):
            mraw = mpool.tile([P, NCOL], u8, tag="mraw")
            nc.sync.dma_start(out=mraw, in_=m[b])
            # u8 -> bf16 label copy on ACT (labels 0..7 exact); keeps
            # DVE free for the per-class stt stream.
            mf = mfpool.tile([P, NCOL], bf16, tag="mf")
            nc.scalar.activation(
                out=mf, in_=mraw, func=mybir.ActivationFunctionType.Copy
            )

            # den2 for all 8 classes up-front: depends only on the mask,
            # so DVE does these while x is still streaming in.
            for c in range(C):
                col = b * C + c
                eq = epool.tile([P, NCOL], bf16, tag="eq")
                nc.vector.tensor_scalar(
                    out=eq,
                    in0=mf,
                    scalar1=float(c),
                    scalar2=None,
                    op0=mybir.AluOpType.is_equal,
                )
                for j in range(NCHUNK):
                    nc.tensor.matmul(
                        out=ps2[:, :],
                        lhsT=etab[col],
                        rhs=eq[:, j * CHUNK : (j + 1) * CHUNK],
                        start=(col == 0 and j == 0),
                        stop=(col == ROWS - 1 and j == NCHUNK - 1),
                    )

            # x stream: staircased so compute starts after 1 MiB and the
            # post-stream compute tail is a single class.
            groups = [1, 3, G] if b == 0 else [G, 3, 1]
            c0 = 0
            for gsz in groups:
                xt = xpool.tile([P, G, NCOL], f32, tag="xt")
                nc.sync.dma_start(
                    out=xt[:, 0:gsz, :],
                    in_=x[b * C + c0 : b * C + c0 + gsz].transpose([1, 0, 2]),
                )
                for i in range(gsz):
                    c = c0 + i
                    col = b * C + c
                    # num partial: (mask == c) * x, accumulated per partition
                    junk = jpool.tile([P, NCOL], mybir.dt.float8e4, tag="jd")
                    nc.vector.scalar_tensor_tensor(
                        out=junk,
                        in0=mf,
                        scalar=float(c),
                        in1=xt[:, i, :],
                        op0=mybir.AluOpType.is_equal,
                        op1=mybir.AluOpType.mult,
                        accum_out=p_num[:, col : col + 1],
                    )
                    # den1 partial: x^2, accumulated per partition
                    sjunk = spool.tile([P, NCOL], mybir.dt.float8e4, tag="ja")
                    nc.scalar.activation(
                        out=sjunk,
                        in_=xt[:, i, :],
                        func=mybir.ActivationFunctionType.Square,
                        accum_out=p_den1[:, col : col + 1],
                    )
                c0 += gsz

        # Fold partition dim of num/den1: [128, 16] -> PSUM [1, 32].
        ps = pspool.tile([1, 2 * ROWS], f32, tag="ps")
        nc.tensor.matmul(out=ps[:, 0:ROWS], lhsT=ones32, rhs=p_num[:], start=True, stop=True)
        nc.tensor.matmul(out=ps[:, ROWS:], lhsT=ones32, rhs=p_den1[:], start=True, stop=True)
        nds = acc.tile([1, 2 * ROWS], f32, tag="nds")
        nc.vector.tensor_copy(out=nds, in_=ps[:])
        nc.sync.dma_start(out=nd[:], in_=nds)

        # den2: fold the chunk dim of all 16 rows at once.
        d2col = acc.tile([ROWS, 1], f32, tag="d2col")
        nc.vector.tensor_reduce(
            out=d2col, in_=ps2[:], axis=mybir.AxisListType.X, op=mybir.AluOpType.add
        )
        nc.sync.dma_start(out=d2[:], in_=d2col)

    nc.compile()
    return nc


def _get(mask64: bool = False) -> bass.Bass:
    if "k" not in _cache:
        _cache["k"] = _build()
    return _cache["k"]


def make_in_maps(output: np.ndarray, mask: np.ndarray, mask64: bool = False):
    # Labels are 0..7: ship the mask as uint8 (lossless) to cut its DMA 4x.
    m8 = mask.astype(np.uint8)
    in_maps = []
    for i in range(N_CORES):
        xs = output[i * B_LOC : (i + 1) * B_LOC].reshape(ROWS, P, NCOL)
        ms = m8[i * B_LOC : (i + 1) * B_LOC].reshape(B_LOC, P, NCOL)
        in_maps.append(
            {"x": np.ascontiguousarray(xs), "m": np.ascontiguousarray(ms)}
        )
    return in_maps


def kernel(output: np.ndarray, mask: np.ndarray) -> np.ndarray:
    global last_results
    output = np.ascontiguousarray(np.asarray(output, dtype=np.float32))
    mask = np.asarray(mask)
    assert output.shape == (B, C, H, W), output.shape
    assert mask.shape == (B, H, W), mask.shape

    nc = _get()
    in_maps = make_in_maps(output, mask)
    last_results = run_bass_kernel_spmd(
        nc,
        in_maps,
        list(range(N_CORES)),
        trace=bool(os.environ.get("DICE_TRACE")),
    )
    # Unshard: dice over the gathered per-(b,c) partials, then the
    # 1 - 2*sum/B^2 affine.
    total = 0.0
    for r in last_results.results:
        nd_ = np.asarray(r["nd"], dtype=np.float64).reshape(2, ROWS)
        d2_ = np.asarray(r["d2"], dtype=np.float64).reshape(ROWS)
        num, den1 = nd_[0], nd_[1]
        total += float(np.sum((num + EPS) / (den1 + d2_ + EPS)))
    loss = 1.0 - 2.0 * total / (B * B)
    return np.float32(loss).reshape(())
